# revision 48
# baseline (speedup 1.0000x reference)
"""MultiHeadGAT layer on 8 trn2 NeuronCores, data-parallel over batch.

Per core (one batch element), with softmax-invariant rescaling: dividing the
unnormalized attention P[j,i] = exp(leaky_relu(e_src[i]+e_dst[j])) by
exp(e_src[i]) (a per-i factor that cancels in the softmax) gives

  P'[j,i] = adj[j,i] * max( s0[j] * gb[i],  s1[j] )

with gb = exp(-0.8*e_src), s0 = exp(0.2*e_dst), s1 = exp(e_dst) -- no exps
in the main loop at all.  The whole per-(head, row-block) elementwise step
runs as ONE custom DVE instruction SCALE_MAX_MUL_ANT
(out = max(in0*s0, s1)*in1) with a HAND-AUTHORED 2X_1PORT uop program:
the stock custom-DVE path is 1x-only, so this kernel writes the 2x table
variant itself (lo element via SRC_0/SRC_1 on datapath blocks 0-2, hi via
SRC_0_HI/SRC_1_HI on delay lanes through blocks 3-5, packed WR0_LO/WR0_HI)
and ORs the byte-36 perf_max bit into the encoded instruction post-codegen.
Measured: [128,1024] bf16 = ~724ns/op vs ~1027ns for the previous
tensor_scalar + per-pair mask tensor_tensor split; adjacency needs no
free-dim duplication, halving its DMA.  AV matmul in bf16 with a ones
column appended to the lhsT so row 64 of the accumulator is the softmax
denominator.

Schedule notes (all measured on HW):
 - No input data is consumable before ~9us (fixed runtime startup) and each
   dma_start on one engine adds ~0.7us of arrival stagger, so the issue
   ORDER is the priority order: adjT[0] (gates the first fused op), then
   the pre chunks (wa slivers -> hT c-halves -> wk) feeding the E_T chain,
   then adjT[1..7].
 - e_src broadcast: heads 0-3 via PE one-hot-selector matmuls (low latency),
   heads 4-7 via four parallel in-SBUF DMA log-doubling chains.
 - Per-pair epilogue DVE work (reciprocals, divides) is deferred past the
   NEXT pair's 4th row-block so the in-order DVE queue never stalls on the
   acc->ACT-copy->PE-transpose chain; non-last pairs divide on ACT, the
   last pair splits divides DVE/ACT since DVE idles there.
 - The first two h0 fused ops are hoisted before h1's broadcast and the
   late e_sb copies run on ACT, so the in-order DVE queue reaches the
   first fused op ~3us sooner.
 - Output staged head-major ([128, H, NB, 64] f32) so each head's flush is
   one DMA of 2KB-contiguous rows (128 descriptors); host un-permutes.

Host-side prep (layout/dtype only): [W@A | h.T | W] packed (wa first so
the highest-priority DMA chunk carries it) and adj.T as bf16, output
un-permutation.  ~74us HW vs 96.8us for the tensor_scalar+mask version
and 227us for the fp32 ACT-exp baseline.
"""
import sys

sys.path.insert(0, "/opt/trn_rl_repo")

import numpy as np
import ml_dtypes

import concourse.bass as bass
import concourse.mybir as mybir
import concourse.tile as tile
import concourse.dve_ops as dve_ops
from concourse.bass_utils import run_bass_kernel_spmd
from concourse.masks import make_identity
from concourse.dve_spec import Spec, Src0, Src1, C0, C1, maxx, lower as dve_lower
from concourse.dve_uop import (
    AluInp as DAluInp,
    AluOp as DAluOp,
    DelayInp as DDelayInp,
    DveOpSpec,
    InpSel as DInpSel,
    OutPath as DOutPath,
    OutSel as DOutSel,
    Trigger as DTrigger,
    UopConfig as DUopConfig,
)
from concourse.library_overlay import lower_extended_insts

F32 = mybir.dt.float32
BF16 = mybir.dt.bfloat16
AF = mybir.ActivationFunctionType
ALU = mybir.AluOpType
BF16NP = ml_dtypes.bfloat16

N_CORES = 8
N = 1024
NB = 8          # row blocks of 128
FIN = 256
KT = 2          # FIN / 128
FO = 512        # heads * fo
H = 8
FOH = 64
ALPHA = 0.2

NSEL = 4        # heads 0-3 via PE selector; 4-7 via parallel DMA doubling

_MAX_SYNC_WAITS = 1



# ---- custom DVE op: out = max(in0*s0, s1)*in1 in ONE 2x pass ------------- #
# The stock path (dve_spec.lower) only emits a 1x uop program and the rust
# encoder hardcodes byte-36 perf_max=0.  The per-NEFF table writer already
# supports perf-mode variant slots, so we hand-author the 2X_1PORT program
# (lo element via SRC_0/SRC_1 on blocks 0-2, hi via SRC_0_HI/SRC_1_HI routed
# on delay lanes through blocks 3-5, results packed WR0_LO/WR0_HI) and OR
# the perf bit into the encoded instruction bytes after lower_extended_insts.
# Measured on HW: [128,1024] bf16 = ~724ns vs 456+571 for the unfused
# tensor_scalar + half of the pair's mask tensor_tensor.

_FUSED_NAME = "SCALE_MAX_MUL_ANT"


def _fused_ref(in0, in1, s0, s1, imm2):
    return (np.maximum(in0.astype(np.float32) * s0, s1) * in1).astype(np.float32)


def _fused_2x_uop():
    u = DUopConfig()
    u.enable_input(DInpSel.SRC_0, 1)
    u.enable_input(DInpSel.CONST_0, 2)
    u.enable_input(DInpSel.CONST_1, 3)
    u.enable_input(DInpSel.SRC_1, 4)
    u.enable_input(DInpSel.SRC_0_HI, 5)
    u.enable_input(DInpSel.SRC_1_HI, 6)
    u.require_inp0 = 1
    u.require_inp1 = 1
    u.trigger = (DTrigger.SRC_TENSOR_DONE, DTrigger.NONE, DTrigger.NONE)
    b = u.datapath_config
    b[0].enable_alu(DAluOp.MULTIPLY, DAluInp.PREV_DELAY_0, DAluInp.PREV_DELAY_1)
    b[0].pass_through_delay(1, 2, 3, 4, 5)
    b[1].enable_alu(DAluOp.MAX, DAluInp.PREV_ALU_OUT, DAluInp.PREV_DELAY_2)
    b[1].pass_through_delay(1, 2, 3, 4, 5)
    b[2].enable_alu(DAluOp.MULTIPLY, DAluInp.PREV_ALU_OUT, DAluInp.PREV_DELAY_3)
    b[2].pass_through_delay(1, 2, 4, 5)
    b[3].enable_alu(DAluOp.MULTIPLY, DAluInp.PREV_DELAY_4, DAluInp.PREV_DELAY_1)
    b[3].enable_delay_from_src(DDelayInp.PREV_ALU_OUT, 0)
    b[3].pass_through_delay(2, 5)
    b[4].enable_alu(DAluOp.MAX, DAluInp.PREV_ALU_OUT, DAluInp.PREV_DELAY_2)
    b[4].pass_through_delay(0, 5)
    b[5].enable_alu(DAluOp.MULTIPLY, DAluInp.PREV_ALU_OUT, DAluInp.PREV_DELAY_5)
    b[5].pass_through_delay(0)
    b[6].pass_through_alu()
    b[6].pass_through_delay(0)
    b[7].pass_through_alu()
    b[7].pass_through_delay(0)
    u.enable_output(DOutSel.DELAY_0, DOutPath.WR0_LO)
    u.enable_output(DOutSel.ALU_OUT, DOutPath.WR0_HI)
    return u


class _FusedOp:
    """Duck-typed dve_ops.DveOp carrying the hand-built 2x variant."""

    def __init__(self):
        self.name = _FUSED_NAME
        self.spec = Spec(body=maxx(Src0 * C0, C1) * Src1, reference=_fused_ref)
        self.subdim = False
        self._cache = {}

    def compile(self, ver):
        if ver not in self._cache:
            s = DveOpSpec(
                name=self.name,
                opcode=dve_ops.get_dve_sub_opcode(self.name),
                uops=dve_lower(self.spec, ver=ver),
                uops_2x=[_fused_2x_uop()],
                rd1_en=True,
                perf_max=1,
            )
            s.validate(ver)
            self._cache[ver] = s
        return self._cache[ver]


_FUSED_OP = None


def _fused_register():
    global _FUSED_OP
    if _FUSED_OP is None:
        op = _FusedOp()
        if _FUSED_NAME not in dve_ops._SUB_OPCODE_FOR_NAME:
            row = dve_ops._CUSTOM_DVE_ROW_BASE + len(dve_ops.OPS)
            assert row < 0x20, row
            dve_ops.OPS.append(op)
            dve_ops._SUB_OPCODE_FOR_NAME[_FUSED_NAME] = row
        _FUSED_OP = op
    return _FUSED_OP


def scale_max_mul(nc, out, in0, in1, s0, s1):
    op = _fused_register()
    return nc.vector._custom_dve(op, out=out, in0=in0, in1=in1, s0=s0, s1=s1)


def _patch_perf_bits(nc):
    """OR byte-36 bit 6 (perf_max=1 -> 2X_1PORT) into every fused-op
    instruction; must run after lower_extended_insts encodes .instr."""
    n = 0
    for f in nc.m.functions:
        for bb in f.blocks:
            for inst in bb.instructions:
                if getattr(inst, "op_name", None) == _FUSED_NAME:
                    raw = list(inst.instr)
                    assert len(raw) > 36 and raw[36] & 0x1F, (len(raw),)
                    raw[36] |= 0x40
                    inst.instr = raw
                    n += 1
    assert n > 0, "no fused instructions found to patch"
    return n


def _split_sync_waits(nc, max_waits=_MAX_SYNC_WAITS):
    """This walrus build rejects instructions carrying more than one sync
    wait; hoist extras onto NOPs inserted just before, on the same engine."""
    uid = 0
    for f in nc.m.functions:
        for bb in f.blocks:
            out = []
            for inst in bb.instructions:
                si = getattr(inst, "sync_info", None)
                if si is not None and si.on_wait and len(si.on_wait) > max_waits:
                    waits = list(si.on_wait)
                    keep = waits[-max_waits:]
                    extra = waits[:-max_waits]
                    si.on_wait.clear()
                    si.on_wait.extend(keep)
                    while extra:
                        chunk, extra = extra[:max_waits], extra[max_waits:]
                        nop = mybir.InstNoOp(
                            name=f"waitsplit-{uid}",
                            engine=inst.engine,
                            sync_info=mybir.SyncInfo(
                                on_wait=list(chunk), on_update=[]
                            ),
                            bass_nofuse=True,
                        )
                        uid += 1
                        out.append(nop)
                out.append(inst)
            bb.instructions[:] = out


def build_nc(split=True):
    nc = bass.Bass()
    PREW = N + 2 * H + FO   # WAb | hTb | Wb packed along the free dim
    pre_d = nc.declare_dram_parameter("pre", [FIN, PREW], BF16, isOutput=False)
    adjT_d = nc.declare_dram_parameter("adjT", [N, N], BF16, isOutput=False)
    # out stored [p, cb, hcol]: row cb*128+p of the logical output lives at
    # out_d[p, cb, :]; the host undoes this with a reshape/transpose
    out_d = nc.declare_dram_parameter("out", [128, H, NB, FOH], F32,
                                      isOutput=True)

    with tile.TileContext(nc) as tc:
        with (
            tc.tile_pool(name="const", bufs=1) as const,
            tc.tile_pool(name="persist", bufs=1) as persist,
            tc.tile_pool(name="tp8", bufs=10) as tpool,
            tc.tile_pool(name="epi", bufs=4) as epi,
            tc.tile_pool(name="psS", bufs=4, space="PSUM") as psS,
            tc.tile_pool(name="psAcc", bufs=1, space="PSUM") as psAcc,
        ):
            ident = const.tile([128, 128], F32, tag="ident")
            make_identity(nc, ident[:])

            pre = [persist.tile([128, PREW], BF16, tag=f"pre{k}",
                                name=f"pre{k}")
                   for k in range(KT)]
            # adjacency (transposed, bf16; the fused DVE op reads it per
            # head so no duplication is needed).  adjT[0] is issued FIRST:
            # it gates the first fused op, and each sync dma_start adds
            # ~0.7us of arrival stagger.
            adjT = [persist.tile([128, N], BF16, tag=f"adjT{j}",
                                 name=f"adjT{j}")
                    for j in range(NB)]
            # chunk boundaries follow need order: [wa|hT c0], [hT c1],
            # adjT0, [wk], adjT1..7 (each dma_start adds ~0.7us of arrival
            # stagger, so issue order = priority order)
            CW = 2 * H + 512
            for k in range(KT):
                nc.sync.dma_start(pre[k][:, 0:CW],
                                  pre_d[k * 128:(k + 1) * 128, 0:CW])
            for k in range(KT):
                nc.sync.dma_start(pre[k][:, CW:CW + 512],
                                  pre_d[k * 128:(k + 1) * 128, CW:CW + 512])
            nc.sync.dma_start(adjT[0][:], adjT_d[0:128, :])
            for k in range(KT):
                nc.sync.dma_start(pre[k][:, CW + 512:PREW],
                                  pre_d[k * 128:(k + 1) * 128, CW + 512:PREW])
            wa = [pre[k][:, 0:2 * H] for k in range(KT)]
            hT = [pre[k][:, 2 * H:2 * H + N] for k in range(KT)]
            wk = [pre[k][:, 2 * H + N:PREW] for k in range(KT)]
            for jb in range(1, NB):
                nc.sync.dma_start(
                    adjT[jb][:], adjT_d[jb * 128:(jb + 1) * 128, :]
                )

            # ---- E_T[16, i] = (WA.T @ hT): rows 0..7 e_src, 8..15 e_dst;
            # G8 = exp(-(1-alpha)*e_src) read straight from PSUM.  Two
            # half-tiles so jb<4 transposes only wait on the c=0 half. ----
            e_tc = [const.tile([16, 512], F32, tag=f"eT{c}", name=f"eT{c}")
                    for c in range(2)]
            g8 = const.tile([8, N], BF16, tag="g8")
            eT_ps = {}
            for c in range(2):
                ps = psS.tile([16, 512], F32, tag="ps")
                eT_ps[c] = ps
                for k in range(KT):
                    nc.tensor.matmul(
                        ps[:], wa[k], hT[k][:, c * 512:(c + 1) * 512],
                        start=(k == 0), stop=(k == KT - 1),
                    )
                nc.scalar.activation(
                    g8[:, c * 512:(c + 1) * 512], ps[0:8, :], AF.Exp,
                    scale=-(1.0 - ALPHA),
                )
            # e_tc[0] feeds esb(0..3) now; e_tc[1] (esb 4..7) is copied
            # later so it doesn't block the first fused ops on DVE
            nc.vector.tensor_copy(e_tc[0][:], eT_ps[0][:])

            # ---- e_sb[jb][p, 16] = E_T[:, jb*128+p]; s0/s1 = per-j scalars ----
            e_sb = [persist.tile([128, 16], F32, tag=f"E{j}", name=f"E{j}")
                    for j in range(NB)]
            s0sb = [persist.tile([128, H], F32, tag=f"s0{j}", name=f"s0{j}")
                    for j in range(NB)]
            s1sb = [persist.tile([128, H], F32, tag=f"s1{j}", name=f"s1{j}")
                    for j in range(NB)]
            def esb(jb, on_act=False):
                tp = psS.tile([128, 512], F32, tag="ps")
                nc.tensor.transpose(
                    tp[:, 0:16],
                    e_tc[jb // 4][:, (jb % 4) * 128:(jb % 4 + 1) * 128],
                    ident[0:16, 0:16],
                )
                if on_act:
                    nc.scalar.copy(e_sb[jb][:], tp[:, 0:16])
                else:
                    nc.vector.tensor_copy(e_sb[jb][:], tp[:, 0:16])

            esb(0)

            def late_esb():
                for jb in range(2, NB):
                    esb(jb, on_act=True)

            # ---- Gb broadcast over partitions via PE selector matmuls.
            # Emission order feeds pair 0 first: heads 0-1, then jb=0 s-cols,
            # then the rest -- PE and ACT are otherwise idle here. ----
            gbsel = [persist.tile([128, N], BF16, tag=f"gb{hh}", name=f"gb{hh}")
                     for hh in range(H)]
            sel = []
            for hh in range(NSEL):
                t = const.tile([8, 128], BF16, tag=f"sel{hh}", name=f"sel{hh}")
                nc.gpsimd.memset(t[:], 0.0)
                nc.gpsimd.affine_select(
                    out=t[:], in_=t[:], pattern=[[0, 128]],
                    compare_op=ALU.not_equal, fill=1.0,
                    base=-hh, channel_multiplier=1,
                )
                sel.append(t)

            def bcast_head(hh, split=False):
                # split=True: c=1 copy on DVE (idle during the prologue) so
                # the two psum->SBUF copies run in parallel with ACT's
                for c in range(2):
                    ps = psS.tile([128, 512], F32, tag="ps")
                    nc.tensor.matmul(
                        ps[:], sel[hh][:], g8[:, c * 512:(c + 1) * 512],
                        start=True, stop=True,
                    )
                    if split:
                        nc.vector.tensor_copy(
                            gbsel[hh][:, c * 512:(c + 1) * 512], ps[:]
                        )
                    else:
                        nc.scalar.copy(
                            gbsel[hh][:, c * 512:(c + 1) * 512], ps[:]
                        )

            def scols(jb):
                # s0 = exp(alpha * e_dst), s1 = exp(e_dst)
                nc.scalar.activation(
                    s0sb[jb][:], e_sb[jb][:, 8:16], AF.Exp, scale=ALPHA,
                )
                nc.scalar.activation(
                    s1sb[jb][:], e_sb[jb][:, 8:16], AF.Exp, scale=1.0,
                )

            scols(0)
            bcast_head(0, split=True)
            esb(1)
            scols(1)
            # first two h0 fused ops hoisted here: everything they need is
            # ready ~2us before h1's broadcast casts clear the DVE queue
            prefill = {}
            for jb in range(2):
                t = tpool.tile([128, 2 * N], BF16, tag="t2",
                               name=f"t2w{jb}")
                prefill[jb] = t
                scale_max_mul(
                    nc, t[:, 0:N], gbsel[0][:, :], adjT[jb][:],
                    s0sb[jb][:, 0:1], s1sb[jb][:, 0:1],
                )
            bcast_head(1, split=True)
            nc.vector.tensor_copy(e_tc[1][:], eT_ps[1][:])
            late_esb()
            for hh in range(NSEL, H):
                t = gbsel[hh]
                nc.sync.dma_start(t[0:1, :], g8[hh:hh + 1, :])
                p = 1
                while p < 128:
                    nc.sync.dma_start(t[p:2 * p, :], t[0:p, :])
                    p *= 2
            for jb in range(2, NB):
                scols(jb)

            def gb(hh):
                return gbsel[hh][:, :]

            # ---- wh_aug[jb][j, h, 0:64] = (h @ W) block bf16, [:, h, 64] = 1 ----
            wh_aug = [persist.tile([128, H, 65], BF16, tag=f"wha{j}",
                                   name=f"wha{j}")
                      for j in range(NB)]
            for jb in range(NB):
                ps = psS.tile([128, H, FOH], F32, tag="ps")
                for k in range(KT):
                    nc.tensor.matmul(
                        ps[:, :, :], hT[k][:, jb * 128:(jb + 1) * 128], wk[k],
                        start=(k == 0), stop=(k == KT - 1),
                    )
                nc.scalar.activation(
                    wh_aug[jb][:, :, 0:64], ps[:, :, :], AF.Copy,
                )
                nc.gpsimd.memset(wh_aug[jb][:, :, 64:65], 1.0)
            for hh in range(2, NSEL):
                bcast_head(hh)

            # ---- output staging: osm_big[p, cb, h*64+f] ----
            osm_big = persist.tile([128, H, NB, FOH], F32, tag="osm")

            # ---- main attention loop, head pairs ----
            # Epilogue DVE work (recip + last-pair osm) is deferred until the
            # next pair's first jb tiles are queued, so the in-order DVE queue
            # never stalls on the acc->ACT->PE transpose chain.
            pending = [None]

            def emit_pending():
                if pending[0] is not None:
                    pending[0]()
                    pending[0] = None

            for hp in range(H // 2):
                h0, h1 = 2 * hp, 2 * hp + 1
                acc = {
                    (hh, c): psAcc.tile([65, 512], F32, tag=f"acc{hh % 2}{c}",
                                        name=f"acc{hh % 2}{c}")
                    for hh in (h0, h1) for c in range(2)
                }
                t2s = {}

                def fused(jb, q, hh):
                    scale_max_mul(
                        nc, t2s[jb][:, q * N:(q + 1) * N], gb(hh), adjT[jb][:],
                        s0sb[jb][:, hh:hh + 1], s1sb[jb][:, hh:hh + 1],
                    )

                def av_mm(jb, q, hh, start, stop):
                    for c in range(2):
                        nc.tensor.matmul(
                            acc[(hh, c)][:],
                            wh_aug[jb][:, hh, :],
                            t2s[jb][:, q * N + c * 512:q * N + (c + 1) * 512],
                            start=start, stop=stop,
                        )

                # epilogue helpers: acc -> SBUF (ACT) + PE transposes,
                # then recips/divides and one head-major flush per head
                # (2KB-contiguous rows, 128 descriptors)
                def build_head(hh, copies_on_dve=False):
                    acc_sb = epi.tile([65, N], F32, tag="accsb")
                    rec8 = epi.tile([128, 8], F32, tag="rec8",
                                    name=f"rec8_{hh}")
                    tps = {}
                    for q in range(2):
                        if copies_on_dve:
                            nc.vector.tensor_copy(
                                acc_sb[:, q * 512:(q + 1) * 512],
                                acc[(hh, q)][:]
                            )
                        else:
                            nc.scalar.copy(
                                acc_sb[:, q * 512:(q + 1) * 512],
                                acc[(hh, q)][:]
                            )
                        tp = psS.tile([128, 4 * 65], F32, tag="ps")
                        for r in range(4):
                            cb = q * 4 + r
                            nc.tensor.transpose(
                                tp[:, r * 65:r * 65 + 65],
                                acc_sb[:, cb * 128:(cb + 1) * 128],
                                ident[0:65, 0:65],
                            )
                        tps[q] = tp
                    return rec8, tps

                def emit_head(hh, rec8, tps, split_dve=False):
                    for q in range(2):
                        tp = tps[q]
                        nc.vector.reciprocal(
                            rec8[:, q * 4:(q + 1) * 4], tp[:, 64::65]
                        )
                        for r in range(4):
                            cb = q * 4 + r
                            dst = osm_big[:, hh, cb, :]
                            if split_dve and r % 2 == 0:
                                nc.vector.tensor_scalar(
                                    dst, tp[:, r * 65:r * 65 + 64],
                                    rec8[:, cb:cb + 1], None, ALU.mult,
                                )
                            else:
                                nc.scalar.activation(
                                    dst, tp[:, r * 65:r * 65 + 64],
                                    AF.Copy, scale=rec8[:, cb:cb + 1],
                                )
                    nc.sync.dma_start(
                        out_d[:, hh, :, :], osm_big[:, hh, :, :],
                    )

                last = hp == H // 2 - 1
                if hp == 0:
                    # jb 0/1 h0 tiles were computed in the prologue
                    t2s.update(prefill)
                if not last:
                    for jb in range(NB):
                        if jb not in t2s:
                            t2s[jb] = tpool.tile([128, 2 * N], BF16,
                                                 tag="t2", name=f"t2_{jb}")
                            fused(jb, 0, h0)
                        fused(jb, 1, h1)
                        for q, hh in enumerate((h0, h1)):
                            av_mm(jb, q, hh, jb == 0, jb == NB - 1)
                        if jb == 3:
                            emit_pending()
                else:
                    for jb in range(NB):
                        t2s[jb] = tpool.tile([128, 2 * N], BF16, tag="t2",
                                             name=f"t2_{jb}")
                        fused(jb, 0, h0)
                        fused(jb, 1, h1)
                        for q, hh in enumerate((h0, h1)):
                            av_mm(jb, q, hh, jb == 0, jb == NB - 1)
                        if jb == 3:
                            emit_pending()
                if not last:
                    parts = [(hh,) + build_head(hh) for hh in (h0, h1)]

                    def emit_epilogue(parts=parts):
                        for hh, rec8, tps in parts:
                            emit_head(hh, rec8, tps)

                    pending[0] = emit_epilogue
                else:
                    parts = [(hh,) + build_head(hh) for hh in (h0, h1)]
                    for i, (hh, rec8, tps) in enumerate(parts):
                        emit_head(hh, rec8, tps, split_dve=True)

    if split:
        _split_sync_waits(nc)
    lower_extended_insts(nc)
    _patch_perf_bits(nc)
    return nc


_NC_CACHE = None


def _get_nc():
    global _NC_CACHE
    if _NC_CACHE is None:
        _NC_CACHE = build_nc()
    return _NC_CACHE


def _prep_in_maps(h, adj, W, a):
    h = np.ascontiguousarray(h, dtype=np.float32)
    adj = np.ascontiguousarray(adj, dtype=np.int32)
    W = np.ascontiguousarray(W, dtype=np.float32)
    a = np.ascontiguousarray(a, dtype=np.float32)
    amat = np.zeros((FO, 2 * H), dtype=np.float32)
    for hh in range(H):
        amat[hh * FOH:(hh + 1) * FOH, hh] = a[hh, :FOH]
        amat[hh * FOH:(hh + 1) * FOH, H + hh] = a[hh, FOH:]
    wamat = (W @ amat).astype(BF16NP)
    wb = W.astype(BF16NP)
    return [
        {
            "pre": np.ascontiguousarray(np.concatenate(
                [wamat, h[c].T.astype(BF16NP), wb], axis=1)),
            "adjT": np.ascontiguousarray(adj[c].T).astype(BF16NP),
        }
        for c in range(N_CORES)
    ]


def run(h, adj, W, a, trace=False, **kw):
    nc = _get_nc()
    in_maps = _prep_in_maps(h, adj, W, a)
    res = run_bass_kernel_spmd(nc, in_maps, list(range(N_CORES)), trace=trace, **kw)
    out = np.stack(
        [res.results[c]["out"].transpose(2, 0, 1, 3).reshape(N, FO)
         for c in range(N_CORES)], axis=0)
    return out.astype(np.float32), res


def kernel(h, adj, W, a):
    out, _ = run(h, adj, W, a)
    if not np.isfinite(out).all():
        # rare first-run flake guard: re-execute once
        out, _ = run(h, adj, W, a)
    return out



# revision 50
# speedup vs baseline: 1.0222x; 1.0222x over previous
"""MultiHeadGAT layer on 8 trn2 NeuronCores, data-parallel over batch.

Per core (one batch element), with softmax-invariant rescaling: dividing the
unnormalized attention P[j,i] = exp(leaky_relu(e_src[i]+e_dst[j])) by
exp(e_src[i]) (a per-i factor that cancels in the softmax) gives

  P'[j,i] = adj[j,i] * max( s0[j] * gb[i],  s1[j] )

with gb = exp(-0.8*e_src), s0 = exp(0.2*e_dst), s1 = exp(e_dst) -- no exps
in the main loop at all.  The whole per-(head, row-block) elementwise step
runs as ONE custom DVE instruction SCALE_MAX_MUL_ANT
(out = max(in0*s0, s1)*in1) with a HAND-AUTHORED 2X_1PORT uop program:
the stock custom-DVE path is 1x-only, so this kernel writes the 2x table
variant itself (lo element via SRC_0/SRC_1 on datapath blocks 0-2, hi via
SRC_0_HI/SRC_1_HI on delay lanes through blocks 3-5, packed WR0_LO/WR0_HI)
and ORs the byte-36 perf_max bit into the encoded instruction post-codegen.
Measured: [128,1024] bf16 = ~724ns/op vs ~1027ns for the previous
tensor_scalar + per-pair mask tensor_tensor split; adjacency needs no
free-dim duplication, halving its DMA.  AV matmul in bf16 with a ones
column appended to the lhsT so row 64 of the accumulator is the softmax
denominator.

Schedule notes (all measured on HW):
 - No input data is consumable before ~9us (fixed runtime startup) and each
   dma_start on one engine adds ~0.7us of arrival stagger, so the issue
   ORDER is the priority order: adjT[0] (gates the first fused op), then
   the pre chunks (wa slivers -> hT c-halves -> wk) feeding the E_T chain,
   then adjT[1..7].
 - e_src broadcast: heads 0-3 via PE one-hot-selector matmuls (low latency),
   heads 4-7 via four parallel in-SBUF DMA log-doubling chains.
 - Per-pair epilogue DVE work (reciprocals, divides) is deferred past the
   NEXT pair's 4th row-block so the in-order DVE queue never stalls on the
   acc->ACT-copy->PE-transpose chain; non-last pairs divide on ACT, the
   last pair splits divides DVE/ACT since DVE idles there.
 - The first two h0 fused ops are hoisted before h1's broadcast and the
   late e_sb copies run on ACT, so the in-order DVE queue reaches the
   first fused op ~3us sooner.
 - Output staged head-major ([128, H, NB, 64] f32) so each head's flush is
   one DMA of 2KB-contiguous rows (128 descriptors); host un-permutes.

Host-side prep (layout/dtype only): [W@A | h.T | W] packed (wa first so
the highest-priority DMA chunk carries it) and adj.T as bf16, output
un-permutation.  ~74us HW vs 96.8us for the tensor_scalar+mask version
and 227us for the fp32 ACT-exp baseline.
"""
import sys

sys.path.insert(0, "/opt/trn_rl_repo")

import numpy as np
import ml_dtypes

import concourse.bass as bass
import concourse.mybir as mybir
import concourse.tile as tile
import concourse.dve_ops as dve_ops
from concourse.bass_utils import run_bass_kernel_spmd
from concourse.masks import make_identity
from concourse.dve_spec import Spec, Src0, Src1, C0, C1, maxx, lower as dve_lower
from concourse.dve_uop import (
    AluInp as DAluInp,
    AluOp as DAluOp,
    DelayInp as DDelayInp,
    DveOpSpec,
    InpSel as DInpSel,
    OutPath as DOutPath,
    OutSel as DOutSel,
    Trigger as DTrigger,
    UopConfig as DUopConfig,
)
from concourse.library_overlay import lower_extended_insts

F32 = mybir.dt.float32
BF16 = mybir.dt.bfloat16
AF = mybir.ActivationFunctionType
ALU = mybir.AluOpType
BF16NP = ml_dtypes.bfloat16

N_CORES = 8
N = 1024
NB = 8          # row blocks of 128
FIN = 256
KT = 2          # FIN / 128
FO = 512        # heads * fo
H = 8
FOH = 64
ALPHA = 0.2

NSEL = 4        # heads 0-3 via PE selector; 4-7 via parallel DMA doubling

_MAX_SYNC_WAITS = 1



# ---- custom DVE op: out = max(in0*s0, s1)*in1 in ONE 2x pass ------------- #
# The stock path (dve_spec.lower) only emits a 1x uop program and the rust
# encoder hardcodes byte-36 perf_max=0.  The per-NEFF table writer already
# supports perf-mode variant slots, so we hand-author the 2X_1PORT program
# (lo element via SRC_0/SRC_1 on blocks 0-2, hi via SRC_0_HI/SRC_1_HI routed
# on delay lanes through blocks 3-5, results packed WR0_LO/WR0_HI) and OR
# the perf bit into the encoded instruction bytes after lower_extended_insts.
# Measured on HW: [128,1024] bf16 = ~724ns vs 456+571 for the unfused
# tensor_scalar + half of the pair's mask tensor_tensor.

_FUSED_NAME = "SCALE_MAX_MUL_ANT"


def _fused_ref(in0, in1, s0, s1, imm2):
    return (np.maximum(in0.astype(np.float32) * s0, s1) * in1).astype(np.float32)


def _fused_2x_uop():
    u = DUopConfig()
    u.enable_input(DInpSel.SRC_0, 1)
    u.enable_input(DInpSel.CONST_0, 2)
    u.enable_input(DInpSel.CONST_1, 3)
    u.enable_input(DInpSel.SRC_1, 4)
    u.enable_input(DInpSel.SRC_0_HI, 5)
    u.enable_input(DInpSel.SRC_1_HI, 6)
    u.require_inp0 = 1
    u.require_inp1 = 1
    u.trigger = (DTrigger.SRC_TENSOR_DONE, DTrigger.NONE, DTrigger.NONE)
    b = u.datapath_config
    b[0].enable_alu(DAluOp.MULTIPLY, DAluInp.PREV_DELAY_0, DAluInp.PREV_DELAY_1)
    b[0].pass_through_delay(1, 2, 3, 4, 5)
    b[1].enable_alu(DAluOp.MAX, DAluInp.PREV_ALU_OUT, DAluInp.PREV_DELAY_2)
    b[1].pass_through_delay(1, 2, 3, 4, 5)
    b[2].enable_alu(DAluOp.MULTIPLY, DAluInp.PREV_ALU_OUT, DAluInp.PREV_DELAY_3)
    b[2].pass_through_delay(1, 2, 4, 5)
    b[3].enable_alu(DAluOp.MULTIPLY, DAluInp.PREV_DELAY_4, DAluInp.PREV_DELAY_1)
    b[3].enable_delay_from_src(DDelayInp.PREV_ALU_OUT, 0)
    b[3].pass_through_delay(2, 5)
    b[4].enable_alu(DAluOp.MAX, DAluInp.PREV_ALU_OUT, DAluInp.PREV_DELAY_2)
    b[4].pass_through_delay(0, 5)
    b[5].enable_alu(DAluOp.MULTIPLY, DAluInp.PREV_ALU_OUT, DAluInp.PREV_DELAY_5)
    b[5].pass_through_delay(0)
    b[6].pass_through_alu()
    b[6].pass_through_delay(0)
    b[7].pass_through_alu()
    b[7].pass_through_delay(0)
    u.enable_output(DOutSel.DELAY_0, DOutPath.WR0_LO)
    u.enable_output(DOutSel.ALU_OUT, DOutPath.WR0_HI)
    return u


class _FusedOp:
    """Duck-typed dve_ops.DveOp carrying the hand-built 2x variant."""

    def __init__(self):
        self.name = _FUSED_NAME
        self.spec = Spec(body=maxx(Src0 * C0, C1) * Src1, reference=_fused_ref)
        self.subdim = False
        self._cache = {}

    def compile(self, ver):
        if ver not in self._cache:
            s = DveOpSpec(
                name=self.name,
                opcode=dve_ops.get_dve_sub_opcode(self.name),
                uops=dve_lower(self.spec, ver=ver),
                uops_2x=[_fused_2x_uop()],
                rd1_en=True,
                perf_max=1,
            )
            s.validate(ver)
            self._cache[ver] = s
        return self._cache[ver]


_FUSED_OP = None


def _fused_register():
    global _FUSED_OP
    if _FUSED_OP is None:
        op = _FusedOp()
        if _FUSED_NAME not in dve_ops._SUB_OPCODE_FOR_NAME:
            row = dve_ops._CUSTOM_DVE_ROW_BASE + len(dve_ops.OPS)
            assert row < 0x20, row
            dve_ops.OPS.append(op)
            dve_ops._SUB_OPCODE_FOR_NAME[_FUSED_NAME] = row
        _FUSED_OP = op
    return _FUSED_OP


def scale_max_mul(nc, out, in0, in1, s0, s1):
    op = _fused_register()
    return nc.vector._custom_dve(op, out=out, in0=in0, in1=in1, s0=s0, s1=s1)


def _patch_perf_bits(nc):
    """OR byte-36 bit 6 (perf_max=1 -> 2X_1PORT) into every fused-op
    instruction; must run after lower_extended_insts encodes .instr."""
    n = 0
    for f in nc.m.functions:
        for bb in f.blocks:
            for inst in bb.instructions:
                if getattr(inst, "op_name", None) == _FUSED_NAME:
                    raw = list(inst.instr)
                    assert len(raw) > 36 and raw[36] & 0x1F, (len(raw),)
                    raw[36] |= 0x40
                    inst.instr = raw
                    n += 1
    assert n > 0, "no fused instructions found to patch"
    return n


def _split_sync_waits(nc, max_waits=_MAX_SYNC_WAITS):
    """This walrus build rejects instructions carrying more than one sync
    wait; hoist extras onto NOPs inserted just before, on the same engine."""
    uid = 0
    for f in nc.m.functions:
        for bb in f.blocks:
            out = []
            for inst in bb.instructions:
                si = getattr(inst, "sync_info", None)
                if si is not None and si.on_wait and len(si.on_wait) > max_waits:
                    waits = list(si.on_wait)
                    keep = waits[-max_waits:]
                    extra = waits[:-max_waits]
                    si.on_wait.clear()
                    si.on_wait.extend(keep)
                    while extra:
                        chunk, extra = extra[:max_waits], extra[max_waits:]
                        nop = mybir.InstNoOp(
                            name=f"waitsplit-{uid}",
                            engine=inst.engine,
                            sync_info=mybir.SyncInfo(
                                on_wait=list(chunk), on_update=[]
                            ),
                            bass_nofuse=True,
                        )
                        uid += 1
                        out.append(nop)
                out.append(inst)
            bb.instructions[:] = out


def build_nc(split=True):
    nc = bass.Bass()
    PREW = N + 2 * H + FO   # WAb | hTb | Wb packed along the free dim
    pre_d = nc.declare_dram_parameter("pre", [FIN, PREW], BF16, isOutput=False)
    adjT_d = nc.declare_dram_parameter("adjT", [N, N], BF16, isOutput=False)
    # out stored [p, cb, hcol]: row cb*128+p of the logical output lives at
    # out_d[p, cb, :]; the host undoes this with a reshape/transpose
    out_d = nc.declare_dram_parameter("out", [128, H, NB, FOH], F32,
                                      isOutput=True)

    with tile.TileContext(nc) as tc:
        with (
            tc.tile_pool(name="const", bufs=1) as const,
            tc.tile_pool(name="persist", bufs=1) as persist,
            tc.tile_pool(name="tp8", bufs=10) as tpool,
            tc.tile_pool(name="epi", bufs=4) as epi,
            tc.tile_pool(name="psS", bufs=4, space="PSUM") as psS,
            tc.tile_pool(name="psAcc", bufs=1, space="PSUM") as psAcc,
        ):
            ident = const.tile([128, 128], F32, tag="ident")
            make_identity(nc, ident[:])

            pre = [persist.tile([128, PREW], BF16, tag=f"pre{k}",
                                name=f"pre{k}")
                   for k in range(KT)]
            # adjacency (transposed, bf16; the fused DVE op reads it per
            # head so no duplication is needed).  adjT[0] is issued FIRST:
            # it gates the first fused op, and each sync dma_start adds
            # ~0.7us of arrival stagger.
            adjT = [persist.tile([128, N], BF16, tag=f"adjT{j}",
                                 name=f"adjT{j}")
                    for j in range(NB)]
            # chunk boundaries follow need order: [wa|hT c0], [hT c1],
            # adjT0, [wk], adjT1..7 (each dma_start adds ~0.7us of arrival
            # stagger, so issue order = priority order)
            CW = 2 * H + 512
            for k in range(KT):
                nc.sync.dma_start(pre[k][:, 0:CW],
                                  pre_d[k * 128:(k + 1) * 128, 0:CW])
            for k in range(KT):
                nc.sync.dma_start(pre[k][:, CW:CW + 512],
                                  pre_d[k * 128:(k + 1) * 128, CW:CW + 512])
            nc.sync.dma_start(adjT[0][:], adjT_d[0:128, :])
            for k in range(KT):
                nc.sync.dma_start(pre[k][:, CW + 512:PREW],
                                  pre_d[k * 128:(k + 1) * 128, CW + 512:PREW])
            wa = [pre[k][:, 0:2 * H] for k in range(KT)]
            hT = [pre[k][:, 2 * H:2 * H + N] for k in range(KT)]
            wk = [pre[k][:, 2 * H + N:PREW] for k in range(KT)]
            for jb in range(1, NB):
                nc.sync.dma_start(
                    adjT[jb][:], adjT_d[jb * 128:(jb + 1) * 128, :]
                )

            # ---- E_T[16, i] = (WA.T @ hT): rows 0..7 e_src, 8..15 e_dst;
            # G8 = exp(-(1-alpha)*e_src) read straight from PSUM.  Two
            # half-tiles so jb<4 transposes only wait on the c=0 half. ----
            e_tc = [const.tile([16, 512], F32, tag=f"eT{c}", name=f"eT{c}")
                    for c in range(2)]
            g8 = const.tile([8, N], BF16, tag="g8")
            eT_ps = {}
            for c in range(2):
                ps = psS.tile([16, 512], F32, tag="ps")
                eT_ps[c] = ps
                for k in range(KT):
                    nc.tensor.matmul(
                        ps[:], wa[k], hT[k][:, c * 512:(c + 1) * 512],
                        start=(k == 0), stop=(k == KT - 1),
                    )
                nc.scalar.activation(
                    g8[:, c * 512:(c + 1) * 512], ps[0:8, :], AF.Exp,
                    scale=-(1.0 - ALPHA),
                )
            # e_tc[0] feeds esb(0..3) now; e_tc[1] (esb 4..7) is copied
            # later so it doesn't block the first fused ops on DVE
            nc.vector.tensor_copy(e_tc[0][:], eT_ps[0][:])

            # ---- e_sb[jb][p, 16] = E_T[:, jb*128+p]; s0/s1 = per-j scalars ----
            e_sb = [persist.tile([128, 16], F32, tag=f"E{j}", name=f"E{j}")
                    for j in range(NB)]
            s0sb = [persist.tile([128, H], F32, tag=f"s0{j}", name=f"s0{j}")
                    for j in range(NB)]
            s1sb = [persist.tile([128, H], F32, tag=f"s1{j}", name=f"s1{j}")
                    for j in range(NB)]
            def esb(jb, on_act=False):
                tp = psS.tile([128, 512], F32, tag="ps")
                nc.tensor.transpose(
                    tp[:, 0:16],
                    e_tc[jb // 4][:, (jb % 4) * 128:(jb % 4 + 1) * 128],
                    ident[0:16, 0:16],
                )
                if on_act:
                    nc.scalar.copy(e_sb[jb][:], tp[:, 0:16])
                else:
                    nc.vector.tensor_copy(e_sb[jb][:], tp[:, 0:16])

            esb(0)

            def late_esb():
                for jb in range(2, NB):
                    esb(jb, on_act=True)

            # ---- Gb broadcast over partitions via PE selector matmuls.
            # Emission order feeds pair 0 first: heads 0-1, then jb=0 s-cols,
            # then the rest -- PE and ACT are otherwise idle here. ----
            gbsel = [persist.tile([128, N], BF16, tag=f"gb{hh}", name=f"gb{hh}")
                     for hh in range(H)]
            sel = []
            for hh in range(NSEL):
                t = const.tile([8, 128], BF16, tag=f"sel{hh}", name=f"sel{hh}")
                nc.gpsimd.memset(t[:], 0.0)
                nc.gpsimd.affine_select(
                    out=t[:], in_=t[:], pattern=[[0, 128]],
                    compare_op=ALU.not_equal, fill=1.0,
                    base=-hh, channel_multiplier=1,
                )
                sel.append(t)

            def bcast_head(hh, split=False):
                # split=True: c=1 copy on DVE (idle during the prologue) so
                # the two psum->SBUF copies run in parallel with ACT's
                for c in range(2):
                    ps = psS.tile([128, 512], F32, tag="ps")
                    nc.tensor.matmul(
                        ps[:], sel[hh][:], g8[:, c * 512:(c + 1) * 512],
                        start=True, stop=True,
                    )
                    if split:
                        nc.vector.tensor_copy(
                            gbsel[hh][:, c * 512:(c + 1) * 512], ps[:]
                        )
                    else:
                        nc.scalar.copy(
                            gbsel[hh][:, c * 512:(c + 1) * 512], ps[:]
                        )

            def scols(jb):
                # s0 = exp(alpha * e_dst), s1 = exp(e_dst)
                nc.scalar.activation(
                    s0sb[jb][:], e_sb[jb][:, 8:16], AF.Exp, scale=ALPHA,
                )
                nc.scalar.activation(
                    s1sb[jb][:], e_sb[jb][:, 8:16], AF.Exp, scale=1.0,
                )

            scols(0)
            bcast_head(0, split=True)
            esb(1)
            scols(1)
            # first two h0 fused ops hoisted here: everything they need is
            # ready ~2us before h1's broadcast casts clear the DVE queue
            prefill = {}
            for jb in range(2):
                t = tpool.tile([128, 2 * N], BF16, tag="t2",
                               name=f"t2w{jb}")
                prefill[jb] = t
                scale_max_mul(
                    nc, t[:, 0:N], gbsel[0][:, :], adjT[jb][:],
                    s0sb[jb][:, 0:1], s1sb[jb][:, 0:1],
                )
            bcast_head(1, split=True)
            nc.vector.tensor_copy(e_tc[1][:], eT_ps[1][:])
            late_esb()
            for hh in range(NSEL, H):
                t = gbsel[hh]
                nc.sync.dma_start(t[0:1, :], g8[hh:hh + 1, :])
                p = 1
                while p < 128:
                    nc.sync.dma_start(t[p:2 * p, :], t[0:p, :])
                    p *= 2
            for jb in range(2, NB):
                scols(jb)

            def gb(hh):
                return gbsel[hh][:, :]

            # ---- wh_aug[jb][j, h, 0:64] = (h @ W) block bf16, [:, h, 64] = 1 ----
            wh_aug = [persist.tile([128, H, 65], BF16, tag=f"wha{j}",
                                   name=f"wha{j}")
                      for j in range(NB)]
            for jb in range(NB):
                ps = psS.tile([128, H, FOH], F32, tag="ps")
                for k in range(KT):
                    nc.tensor.matmul(
                        ps[:, :, :], hT[k][:, jb * 128:(jb + 1) * 128], wk[k],
                        start=(k == 0), stop=(k == KT - 1),
                    )
                nc.scalar.activation(
                    wh_aug[jb][:, :, 0:64], ps[:, :, :], AF.Copy,
                )
                nc.gpsimd.memset(wh_aug[jb][:, :, 64:65], 1.0)
            for hh in range(2, NSEL):
                bcast_head(hh)

            # ---- output staging: osm_big[p, cb, h*64+f] ----
            osm_big = persist.tile([128, H, NB, FOH], F32, tag="osm")

            # ---- main attention loop, head pairs ----
            # Epilogue DVE work (recip + last-pair osm) is deferred until the
            # next pair's first jb tiles are queued, so the in-order DVE queue
            # never stalls on the acc->ACT->PE transpose chain.
            pending = [None]

            def emit_pending():
                if pending[0] is not None:
                    pending[0]()
                    pending[0] = None

            for hp in range(H // 2):
                h0, h1 = 2 * hp, 2 * hp + 1
                acc = {
                    (hh, c): psAcc.tile([65, 512], F32, tag=f"acc{hh % 2}{c}",
                                        name=f"acc{hh % 2}{c}")
                    for hh in (h0, h1) for c in range(2)
                }
                t2s = {}

                def fused(jb, q, hh):
                    scale_max_mul(
                        nc, t2s[jb][:, q * N:(q + 1) * N], gb(hh), adjT[jb][:],
                        s0sb[jb][:, hh:hh + 1], s1sb[jb][:, hh:hh + 1],
                    )

                def av_mm(jb, q, hh, start, stop):
                    for c in range(2):
                        nc.tensor.matmul(
                            acc[(hh, c)][:],
                            wh_aug[jb][:, hh, :],
                            t2s[jb][:, q * N + c * 512:q * N + (c + 1) * 512],
                            start=start, stop=stop,
                        )

                # epilogue helpers: acc -> SBUF (ACT) + PE transposes,
                # then recips/divides and one head-major flush per head
                # (2KB-contiguous rows, 128 descriptors)
                def build_head(hh, copies_on_dve=False):
                    acc_sb = epi.tile([65, N], F32, tag="accsb")
                    rec8 = epi.tile([128, 8], F32, tag="rec8",
                                    name=f"rec8_{hh}")
                    tps = {}
                    for q in range(2):
                        if copies_on_dve:
                            nc.vector.tensor_copy(
                                acc_sb[:, q * 512:(q + 1) * 512],
                                acc[(hh, q)][:]
                            )
                        else:
                            nc.scalar.copy(
                                acc_sb[:, q * 512:(q + 1) * 512],
                                acc[(hh, q)][:]
                            )
                        tp = psS.tile([128, 4 * 65], F32, tag="ps")
                        for r in range(4):
                            cb = q * 4 + r
                            nc.tensor.transpose(
                                tp[:, r * 65:r * 65 + 65],
                                acc_sb[:, cb * 128:(cb + 1) * 128],
                                ident[0:65, 0:65],
                            )
                        tps[q] = tp
                    return rec8, tps

                def emit_head(hh, rec8, tps, split_dve=False,
                              flush_per_q=False):
                    for q in range(2):
                        tp = tps[q]
                        nc.vector.reciprocal(
                            rec8[:, q * 4:(q + 1) * 4], tp[:, 64::65]
                        )
                        for r in range(4):
                            cb = q * 4 + r
                            dst = osm_big[:, hh, cb, :]
                            if split_dve and r % 2 == 0:
                                nc.vector.tensor_scalar(
                                    dst, tp[:, r * 65:r * 65 + 64],
                                    rec8[:, cb:cb + 1], None, ALU.mult,
                                )
                            else:
                                nc.scalar.activation(
                                    dst, tp[:, r * 65:r * 65 + 64],
                                    AF.Copy, scale=rec8[:, cb:cb + 1],
                                )
                        if flush_per_q:
                            nc.sync.dma_start(
                                out_d[:, hh, q * 4:(q + 1) * 4, :],
                                osm_big[:, hh, q * 4:(q + 1) * 4, :],
                            )
                    if not flush_per_q:
                        nc.sync.dma_start(
                            out_d[:, hh, :, :], osm_big[:, hh, :, :],
                        )

                last = hp == H // 2 - 1
                if hp == 0:
                    # jb 0/1 h0 tiles were computed in the prologue
                    t2s.update(prefill)
                if not last:
                    for jb in range(NB):
                        if jb not in t2s:
                            t2s[jb] = tpool.tile([128, 2 * N], BF16,
                                                 tag="t2", name=f"t2_{jb}")
                            fused(jb, 0, h0)
                        fused(jb, 1, h1)
                        for q, hh in enumerate((h0, h1)):
                            av_mm(jb, q, hh, jb == 0, jb == NB - 1)
                        if jb == 3:
                            emit_pending()
                else:
                    for jb in range(NB):
                        t2s[jb] = tpool.tile([128, 2 * N], BF16, tag="t2",
                                             name=f"t2_{jb}")
                        fused(jb, 0, h0)
                        fused(jb, 1, h1)
                        for q, hh in enumerate((h0, h1)):
                            av_mm(jb, q, hh, jb == 0, jb == NB - 1)
                        if jb == 2:
                            emit_pending()
                if not last:
                    parts = [(hh,) + build_head(hh) for hh in (h0, h1)]

                    def emit_epilogue(parts=parts):
                        for hh, rec8, tps in parts:
                            emit_head(hh, rec8, tps)

                    pending[0] = emit_epilogue
                else:
                    parts = [(hh,) + build_head(hh) for hh in (h0, h1)]
                    for i, (hh, rec8, tps) in enumerate(parts):
                        emit_head(hh, rec8, tps, split_dve=True,
                                  flush_per_q=(i == 1))

    if split:
        _split_sync_waits(nc)
    lower_extended_insts(nc)
    _patch_perf_bits(nc)
    return nc


_NC_CACHE = None


def _get_nc():
    global _NC_CACHE
    if _NC_CACHE is None:
        _NC_CACHE = build_nc()
    return _NC_CACHE


def _prep_in_maps(h, adj, W, a):
    h = np.ascontiguousarray(h, dtype=np.float32)
    adj = np.ascontiguousarray(adj, dtype=np.int32)
    W = np.ascontiguousarray(W, dtype=np.float32)
    a = np.ascontiguousarray(a, dtype=np.float32)
    amat = np.zeros((FO, 2 * H), dtype=np.float32)
    for hh in range(H):
        amat[hh * FOH:(hh + 1) * FOH, hh] = a[hh, :FOH]
        amat[hh * FOH:(hh + 1) * FOH, H + hh] = a[hh, FOH:]
    wamat = (W @ amat).astype(BF16NP)
    wb = W.astype(BF16NP)
    return [
        {
            "pre": np.ascontiguousarray(np.concatenate(
                [wamat, h[c].T.astype(BF16NP), wb], axis=1)),
            "adjT": np.ascontiguousarray(adj[c].T).astype(BF16NP),
        }
        for c in range(N_CORES)
    ]


def run(h, adj, W, a, trace=False, **kw):
    nc = _get_nc()
    in_maps = _prep_in_maps(h, adj, W, a)
    res = run_bass_kernel_spmd(nc, in_maps, list(range(N_CORES)), trace=trace, **kw)
    out = np.stack(
        [res.results[c]["out"].transpose(2, 0, 1, 3).reshape(N, FO)
         for c in range(N_CORES)], axis=0)
    return out.astype(np.float32), res


def kernel(h, adj, W, a):
    out, _ = run(h, adj, W, a)
    if not np.isfinite(out).all():
        # rare first-run flake guard: re-execute once
        out, _ = run(h, adj, W, a)
    return out

